# revision 35
# baseline (speedup 1.0000x reference)
"""Trainium2 Bass kernel for nn_FEDformerEncoder (8-core data parallel).

The reference network is, per layer (L=2):
    y  = mean_e( conv1d_same(x, w_e) + b_e )              (depthwise conv on W)
    q,k,v = y @ w{q,k,v}.T + b{q,k,v}                     ([rows, P])
    Q,K,V = fft(q),fft(k),fft(v)
    Wt = K * conj(Q) / sqrt(P) * V
    out = ifft(Wt).real @ wo.T + bo

Everything except the elementwise complex triple product is linear in x, so
the conv, the FFT, and the iFFT fold into host-precomputed projection
weights.  Real-input FFT symmetry packs each 1024-bin complex spectrum into
exactly 1024 reals per signal: block A = Re[0..511], block B =
[Re[512](Nyquist), Im[1..511]].  Composing the two layers' linear maps
(iFFT-projection of layer 1 directly into conv+FFT-projection of layer 2)
collapses the whole network into three matmul stages and two elementwise
stages:

    S1  = x   @ EW1  + b1     # [rows,2048] @ [2048,3072]
    Wt1 = complex-triple(S1)  # packed; slot 0 of A/B = DC/Nyquist, real
    S2  = Wt1 @ M12  + b2     # [rows,1024] @ [1024,3072], M12 = WoP1@EW2
    Wt2 = complex-triple(S2)
    out = Wt2 @ WoP2 + bo2    # [rows,1024] @ [1024,2048]

Sharded batch-wise over 8 cores (4 batches = 512 rows per core), weights
replicated.  Activations live in SBUF in transposed layout [feature(part),
row(free)] throughout, so no on-device transposes are needed.  Matmul
operands are fp16 (fp32 PSUM accumulation, fp32 elementwise); contraction
row-tiles are interleaved [A0 B0 A1 B1 ...] so each elementwise group
feeds the next stage in production order and the stages pipeline.
"""
import sys

import numpy as np

sys.path.insert(0, "/opt/trn_rl_repo")

import concourse.bass as bass
import concourse.mybir as mybir
import concourse.tile as tile
from concourse import bacc
from concourse.bass_utils import run_bass_kernel_spmd

BS, CNT, W, P, E, KK, L = 32, 128, 2048, 1024, 8, 25, 2
H = P // 2                    # 512 slots per packed block
NCORES = 8
ROWS = (BS // NCORES) * CNT   # 512 rows per core
KT = W // 128                 # 16 contraction tiles (stage 1)
MT = (3 * P) // 128           # 24 output tiles (stages 1,2: q|k|v packed)
ST = P // 128                 # 8 contraction tiles (stages 2,3)
WT = W // 128                 # 16 output tiles (stage 3)
F32 = mybir.dt.float32
ACT = mybir.dt.float16
ACT_NP = np.float16
IDENT = mybir.ActivationFunctionType.Identity


def _fold_layer(conv_w, conv_b, wq, bq, wk, bk, wv, bv, wo, bo):
    """Fold conv + FFT into projection weights (float64 math).

    Returns EW [W, 3*P], Sbias [3*P], WoP [P, W] (rows interleaved
    [A0 B0 A1 B1 A2 B2 A3 B3] by 128-tile), bo [W].
    """
    f64 = np.float64
    wbar = conv_w.astype(f64).mean(axis=0)[0]          # [KK]
    bbar = conv_b.astype(f64).mean()

    idx = np.arange(W)
    D = idx[None, :] - idx[:, None] + (KK // 2)        # C[w,u] = wbar[u-w+12]
    C = np.where((D >= 0) & (D < KK), wbar[np.clip(D, 0, KK - 1)], 0.0)

    def pack_fwd(wmat, bvec, scale=1.0):
        Wf = np.fft.fft(wmat.astype(f64), axis=0)      # [P, W]
        Bf = np.fft.fft(bvec.astype(f64))              # [P]
        cols = np.empty((W, P), dtype=f64)
        cols[:, :H] = Wf[:H, :].real.T
        cols[:, H] = Wf[H, :].real
        cols[:, H + 1:] = Wf[1:H, :].imag.T
        bias = np.empty(P, dtype=f64)
        bias[:H] = Bf[:H].real
        bias[H] = Bf[H].real
        bias[H + 1:] = Bf[1:H].imag
        return cols * scale, bias * scale

    s = 1.0 / np.sqrt(f64(P))
    cq, bq_p = pack_fwd(wq, bq)
    ck, bk_p = pack_fwd(wk, bk)
    cv, bv_p = pack_fwd(wv, bv, scale=s)
    cols = np.concatenate([cq, ck, cv], axis=1)        # [W, 3P]
    bias = np.concatenate([bq_p, bk_p, bv_p])

    EW = C.T @ cols
    Sbias = bbar * cols.sum(axis=0) + bias

    G = np.fft.ifft(wo.astype(f64), axis=1)            # [W, P]
    WoP = np.empty((P, W), dtype=f64)
    WoP[0] = G[:, 0].real
    WoP[1:H] = 2.0 * G[:, 1:H].real.T
    WoP[H] = G[:, H].real
    WoP[H + 1:] = -2.0 * G[:, 1:H].imag.T
    # interleave row-tiles A0 B0 A1 B1 ... to match Wcat production order
    WoP = WoP.reshape(2, 4, 128, W).transpose(1, 0, 2, 3).reshape(P, W)

    return EW, Sbias, WoP, bo.astype(f64)


def _build_module():
    nc = bacc.Bacc("TRN2", target_bir_lowering=False, debug=False)

    xin = nc.dram_tensor("xin", [KT // 2, 128, 2 * ROWS], ACT,
                         kind="ExternalInput")
    ew1 = nc.dram_tensor("ew1", [MT, 128, W], ACT, kind="ExternalInput")
    m12 = nc.dram_tensor("m12", [MT, 128, P], ACT, kind="ExternalInput")
    wop2 = nc.dram_tensor("wop2", [WT, 128, P], ACT, kind="ExternalInput")
    # [sb1(24) | sb2(24) | bo2(16)] packed into one [128, 64] tensor
    biases = nc.dram_tensor("biases", [128, 2 * MT + WT], F32,
                            kind="ExternalInput")
    xout = nc.dram_tensor("xout", [WT, 128, ROWS], F32, kind="ExternalOutput")

    with tile.TileContext(nc) as tc:
        with (
            tc.tile_pool(name="xbuf", bufs=KT) as xpool,
            tc.tile_pool(name="bias", bufs=1) as bpool,
            tc.tile_pool(name="wfw", bufs=5) as wfpool,
            tc.tile_pool(name="wm", bufs=6) as wmpool,
            tc.tile_pool(name="wo2", bufs=WT) as wo2pool,
            tc.tile_pool(name="spec", bufs=16) as spool,
            tc.tile_pool(name="wt", bufs=16) as wtpool,
            tc.tile_pool(name="ew", bufs=8) as ewpool,
            tc.tile_pool(name="out", bufs=4) as opool,
            tc.tile_pool(name="psum", bufs=8, space="PSUM") as pspool,
        ):
            # first stage-1 weight blocks go out before x so the PE can
            # start as soon as x k-tiles stream in; x loads issue on the
            # (idle-at-start) scalar engine's DGE, weights on sync's
            # chunked so the first matmuls start after the first 512-col
            # slice lands rather than after the whole 768KB block
            # x pair 0 rides first on the sync queue — it gates the very
            # first matmul and sync's DGE arms earliest
            x0 = xpool.tile([128, 2 * ROWS], ACT, tag="x")
            nc.sync.dma_start(x0[:], xin[0])

            wf_tiles = {}
            for j in (0, 4):                 # t=0 groups' first two blocks
                wtile = wfpool.tile([128, W], ACT, tag="wf")
                for c in range(4):
                    nc.sync.dma_start(wtile[:, bass.ts(c, W // 4)],
                                      ew1[j][:, bass.ts(c, W // 4)])
                wf_tiles[j] = wtile

            xpair = [x0]
            for kp in range(1, KT // 2):
                xt = xpool.tile([128, 2 * ROWS], ACT, tag="x")
                nc.scalar.dma_start(xt[:], xin[kp])
                xpair.append(xt)

            def xslice(k):
                return xpair[k // 2][:, bass.ts(k % 2, ROWS)]

            btile = bpool.tile([128, 2 * MT + WT], F32, tag="biases")
            nc.sync.dma_start(btile[:], biases[:])

            def sb1_col(j):
                return btile[:, j:j + 1]

            def sb2_col(j):
                return btile[:, MT + j:MT + j + 1]

            def bo2_col(j):
                return btile[:, 2 * MT + j:2 * MT + j + 1]

            def elementwise(St, first):
                """complex triple product on one partition-row group."""
                qA, qB, kA, kB, vA, vB = St
                v = nc.vector
                cr = ewpool.tile([128, ROWS], F32, tag="ew")
                ci = ewpool.tile([128, ROWS], F32, tag="ew")
                t0 = ewpool.tile([128, ROWS], F32, tag="ew")
                v.tensor_mul(cr[:], kA[:], qA[:])
                v.tensor_mul(t0[:], kB[:], qB[:])
                v.tensor_add(cr[:], cr[:], t0[:])
                v.tensor_mul(ci[:], kB[:], qA[:])
                v.tensor_mul(t0[:], kA[:], qB[:])
                v.tensor_sub(ci[:], ci[:], t0[:])
                wr = wtpool.tile([128, ROWS], ACT, tag="wt")
                wi = wtpool.tile([128, ROWS], ACT, tag="wt")
                v.tensor_mul(wr[:], cr[:], vA[:])
                v.tensor_mul(t0[:], ci[:], vB[:])
                v.tensor_sub(wr[:], wr[:], t0[:])
                v.tensor_mul(wi[:], cr[:], vB[:])
                v.tensor_mul(t0[:], ci[:], vA[:])
                v.tensor_add(wi[:], wi[:], t0[:])
                if first:
                    # slot 0: A holds DC, B holds Nyquist — both real
                    v.tensor_mul(t0[0:1, :], qA[0:1, :], kA[0:1, :])
                    v.tensor_mul(wr[0:1, :], t0[0:1, :], vA[0:1, :])
                    v.tensor_mul(t0[0:1, :], qB[0:1, :], kB[0:1, :])
                    v.tensor_mul(wi[0:1, :], t0[0:1, :], vB[0:1, :])
                return wr, wi

            # ---- stage 1: S1 = x @ EW1 + b1, pipelined elementwise ----
            # The first two groups' k-loops are interleaved so every
            # arriving x pair unlocks 4 matmuls — keeps the PE fed while
            # x is still streaming in at kernel start.
            Wcat1 = [None] * ST
            for t in range(4):
                St = []
                if t == 0:
                    w0, w1 = wf_tiles.pop(0), wf_tiles.pop(4)
                    ps0 = pspool.tile([128, ROWS], F32, tag="ps")
                    ps1 = pspool.tile([128, ROWS], F32, tag="ps")
                    for k in range(KT):
                        nc.tensor.matmul(
                            ps0[:], w0[:, bass.ts(k, 128)], xslice(k),
                            start=(k == 0), stop=(k == KT - 1))
                        nc.tensor.matmul(
                            ps1[:], w1[:, bass.ts(k, 128)], xslice(k),
                            start=(k == 0), stop=(k == KT - 1))
                    for j, ps in ((0, ps0), (4, ps1)):
                        Sj = spool.tile([128, ROWS], F32, tag="spec")
                        nc.scalar.activation(Sj[:], ps[:], IDENT,
                                             bias=sb1_col(j))
                        St.append(Sj)
                bstart = 2 if t == 0 else 0
                for b in range(bstart, 6):   # qA qB kA kB vA vB row t
                    j = b * 4 + t
                    wtile = wfpool.tile([128, W], ACT, tag="wf")
                    nc.sync.dma_start(wtile[:], ew1[j])
                    ps = pspool.tile([128, ROWS], F32, tag="ps")
                    for k in range(KT):
                        nc.tensor.matmul(
                            ps[:], wtile[:, bass.ts(k, 128)], xslice(k),
                            start=(k == 0), stop=(k == KT - 1))
                    Sj = spool.tile([128, ROWS], F32, tag="spec")
                    nc.scalar.activation(Sj[:], ps[:], IDENT,
                                         bias=sb1_col(j))
                    St.append(Sj)
                wr, wi = elementwise(St, t == 0)
                Wcat1[2 * t] = wr
                Wcat1[2 * t + 1] = wi

            # ---- stage 2: S2 = Wt1 @ M12 + b2, pipelined elementwise ----
            # wop2 is preloaded during stage 2 so stage 3 never starves
            wo2_tiles = []
            Wcat2 = [None] * ST
            for t in range(4):
                St = []
                for b in range(6):
                    j = b * 4 + t
                    if len(wo2_tiles) < WT:
                        j2 = len(wo2_tiles)
                        w2 = wo2pool.tile([128, P], ACT, tag="wo2")
                        nc.scalar.dma_start(w2[:], wop2[j2])
                        wo2_tiles.append(w2)
                    wtile = wmpool.tile([128, P], ACT, tag="wm")
                    nc.sync.dma_start(wtile[:], m12[j])
                    ps = pspool.tile([128, ROWS], F32, tag="ps")
                    for s in range(ST):
                        nc.tensor.matmul(
                            ps[:], wtile[:, bass.ts(s, 128)], Wcat1[s][:],
                            start=(s == 0), stop=(s == ST - 1))
                    Sj = spool.tile([128, ROWS], F32, tag="spec")
                    nc.scalar.activation(Sj[:], ps[:], IDENT,
                                         bias=sb2_col(j))
                    St.append(Sj)
                wr, wi = elementwise(St, t == 0)
                Wcat2[2 * t] = wr
                Wcat2[2 * t + 1] = wi

            # ---- stage 3: out = Wt2 @ WoP2 + bo2 ----
            for j in range(WT):
                wtile = wo2_tiles[j]
                ps = pspool.tile([128, ROWS], F32, tag="ps")
                for s in range(ST):
                    nc.tensor.matmul(
                        ps[:], wtile[:, bass.ts(s, 128)], Wcat2[s][:],
                        start=(s == 0), stop=(s == ST - 1))
                ostage = opool.tile([128, ROWS], F32, tag="out")
                nc.vector.tensor_scalar_add(ostage[:], ps[:], bo2_col(j))
                (nc.scalar if j % 2 else nc.sync).dma_start(
                    xout[j], ostage[:])
    nc.compile()
    return nc


_MODULE_CACHE = {}


def _get_module():
    if "nc" not in _MODULE_CACHE:
        _MODULE_CACHE["nc"] = _build_module()
    return _MODULE_CACHE["nc"]


def _prepare_weight_maps(conv_w, conv_b, wq, bq, wk, bk, wv, bv, wo, bo):
    folds = [_fold_layer(conv_w[l], conv_b[l], wq[l], bq[l], wk[l], bk[l],
                         wv[l], bv[l], wo[l], bo[l]) for l in range(L)]
    EW1, Sb1, WoP1, _bo1 = folds[0]
    EW2, Sb2, WoP2, bo2 = folds[1]
    M12 = WoP1 @ EW2                               # [P, 3P], fp64
    Sb2e = _bo1 @ EW2 + Sb2                        # [3P]

    def pack(Wm, n_k, n_m):
        # [n_k*128, n_m*128] -> [n_m, 128, n_k*128] partition-contiguous
        return np.ascontiguousarray(
            Wm.reshape(n_k, 128, n_m, 128).transpose(2, 1, 0, 3)
            .reshape(n_m, 128, n_k * 128).astype(ACT_NP))

    biases = np.concatenate([
        Sb1.reshape(MT, 128).T, Sb2e.reshape(MT, 128).T,
        bo2.reshape(WT, 128).T], axis=1).astype(np.float32)
    return {
        "ew1": pack(EW1, KT, MT),
        "m12": pack(M12, ST, MT),
        "wop2": pack(WoP2, ST, WT),
        "biases": np.ascontiguousarray(biases),
    }


def _make_in_maps(inputs):
    x = np.asarray(inputs["x"], dtype=np.float32)
    wmap = _prepare_weight_maps(
        np.asarray(inputs["conv_w"]), np.asarray(inputs["conv_b"]),
        np.asarray(inputs["wq"]), np.asarray(inputs["bq"]),
        np.asarray(inputs["wk"]), np.asarray(inputs["bk"]),
        np.asarray(inputs["wv"]), np.asarray(inputs["bv"]),
        np.asarray(inputs["wo"]), np.asarray(inputs["bo"]))
    per_core = BS // NCORES
    in_maps = []
    for c in range(NCORES):
        xc = x[c * per_core:(c + 1) * per_core].reshape(ROWS, W)
        xin = np.ascontiguousarray(
            xc.reshape(ROWS, KT // 2, 2, 128).transpose(1, 3, 2, 0)
            .reshape(KT // 2, 128, 2 * ROWS).astype(ACT_NP))
        in_maps.append({"xin": xin, **wmap})
    return in_maps


def kernel(x, conv_w, conv_b, wq, bq, wk, bk, wv, bv, wo, bo):
    in_maps = _make_in_maps(dict(
        x=x, conv_w=conv_w, conv_b=conv_b, wq=wq, bq=bq, wk=wk, bk=bk,
        wv=wv, bv=bv, wo=wo, bo=bo))
    nc = _get_module()
    res = run_bass_kernel_spmd(nc, in_maps, list(range(NCORES)))

    per_core = BS // NCORES
    outs = []
    for c in range(NCORES):
        xo = res.results[c]["xout"]                    # [WT, 128, ROWS]
        outs.append(xo.transpose(2, 0, 1).reshape(per_core, CNT, W))
    return np.concatenate(outs, axis=0).astype(np.float32)


# revision 36
# speedup vs baseline: 1.0071x; 1.0071x over previous
"""Trainium2 Bass kernel for nn_FEDformerEncoder (8-core data parallel).

The reference network is, per layer (L=2):
    y  = mean_e( conv1d_same(x, w_e) + b_e )              (depthwise conv on W)
    q,k,v = y @ w{q,k,v}.T + b{q,k,v}                     ([rows, P])
    Q,K,V = fft(q),fft(k),fft(v)
    Wt = K * conj(Q) / sqrt(P) * V
    out = ifft(Wt).real @ wo.T + bo

Everything except the elementwise complex triple product is linear in x, so
the conv, the FFT, and the iFFT fold into host-precomputed projection
weights.  Real-input FFT symmetry packs each 1024-bin complex spectrum into
exactly 1024 reals per signal: block A = Re[0..511], block B =
[Re[512](Nyquist), Im[1..511]].  Composing the two layers' linear maps
(iFFT-projection of layer 1 directly into conv+FFT-projection of layer 2)
collapses the whole network into three matmul stages and two elementwise
stages:

    S1  = x   @ EW1  + b1     # [rows,2048] @ [2048,3072]
    Wt1 = complex-triple(S1)  # packed; slot 0 of A/B = DC/Nyquist, real
    S2  = Wt1 @ M12  + b2     # [rows,1024] @ [1024,3072], M12 = WoP1@EW2
    Wt2 = complex-triple(S2)
    out = Wt2 @ WoP2 + bo2    # [rows,1024] @ [1024,2048]

Sharded batch-wise over 8 cores (4 batches = 512 rows per core), weights
replicated.  Activations live in SBUF in transposed layout [feature(part),
row(free)] throughout, so no on-device transposes are needed.  Matmul
operands are fp16 (fp32 PSUM accumulation, fp32 elementwise); contraction
row-tiles are interleaved [A0 B0 A1 B1 ...] so each elementwise group
feeds the next stage in production order and the stages pipeline.
"""
import sys

import numpy as np

sys.path.insert(0, "/opt/trn_rl_repo")

import concourse.bass as bass
import concourse.mybir as mybir
import concourse.tile as tile
from concourse import bacc
from concourse.bass_utils import run_bass_kernel_spmd

BS, CNT, W, P, E, KK, L = 32, 128, 2048, 1024, 8, 25, 2
H = P // 2                    # 512 slots per packed block
NCORES = 8
ROWS = (BS // NCORES) * CNT   # 512 rows per core
KT = W // 128                 # 16 contraction tiles (stage 1)
MT = (3 * P) // 128           # 24 output tiles (stages 1,2: q|k|v packed)
ST = P // 128                 # 8 contraction tiles (stages 2,3)
WT = W // 128                 # 16 output tiles (stage 3)
F32 = mybir.dt.float32
ACT = mybir.dt.float16
ACT_NP = np.float16
IDENT = mybir.ActivationFunctionType.Identity


def _fold_layer(conv_w, conv_b, wq, bq, wk, bk, wv, bv, wo, bo):
    """Fold conv + FFT into projection weights (float64 math).

    Returns EW [W, 3*P], Sbias [3*P], WoP [P, W] (rows interleaved
    [A0 B0 A1 B1 A2 B2 A3 B3] by 128-tile), bo [W].
    """
    f64 = np.float64
    wbar = conv_w.astype(f64).mean(axis=0)[0]          # [KK]
    bbar = conv_b.astype(f64).mean()

    idx = np.arange(W)
    D = idx[None, :] - idx[:, None] + (KK // 2)        # C[w,u] = wbar[u-w+12]
    C = np.where((D >= 0) & (D < KK), wbar[np.clip(D, 0, KK - 1)], 0.0)

    def pack_fwd(wmat, bvec, scale=1.0):
        Wf = np.fft.fft(wmat.astype(f64), axis=0)      # [P, W]
        Bf = np.fft.fft(bvec.astype(f64))              # [P]
        cols = np.empty((W, P), dtype=f64)
        cols[:, :H] = Wf[:H, :].real.T
        cols[:, H] = Wf[H, :].real
        cols[:, H + 1:] = Wf[1:H, :].imag.T
        bias = np.empty(P, dtype=f64)
        bias[:H] = Bf[:H].real
        bias[H] = Bf[H].real
        bias[H + 1:] = Bf[1:H].imag
        return cols * scale, bias * scale

    s = 1.0 / np.sqrt(f64(P))
    cq, bq_p = pack_fwd(wq, bq)
    ck, bk_p = pack_fwd(wk, bk)
    cv, bv_p = pack_fwd(wv, bv, scale=s)
    cols = np.concatenate([cq, ck, cv], axis=1)        # [W, 3P]
    bias = np.concatenate([bq_p, bk_p, bv_p])

    EW = C.T @ cols
    Sbias = bbar * cols.sum(axis=0) + bias

    G = np.fft.ifft(wo.astype(f64), axis=1)            # [W, P]
    WoP = np.empty((P, W), dtype=f64)
    WoP[0] = G[:, 0].real
    WoP[1:H] = 2.0 * G[:, 1:H].real.T
    WoP[H] = G[:, H].real
    WoP[H + 1:] = -2.0 * G[:, 1:H].imag.T
    # interleave row-tiles A0 B0 A1 B1 ... to match Wcat production order
    WoP = WoP.reshape(2, 4, 128, W).transpose(1, 0, 2, 3).reshape(P, W)

    return EW, Sbias, WoP, bo.astype(f64)


def _build_module():
    nc = bacc.Bacc("TRN2", target_bir_lowering=False, debug=False)

    xin = nc.dram_tensor("xin", [KT // 2, 128, 2 * ROWS], ACT,
                         kind="ExternalInput")
    ew1 = nc.dram_tensor("ew1", [MT, 128, W], ACT, kind="ExternalInput")
    m12 = nc.dram_tensor("m12", [MT, 128, P], ACT, kind="ExternalInput")
    wop2 = nc.dram_tensor("wop2", [WT, 128, P], ACT, kind="ExternalInput")
    # [sb1(24) | sb2(24) | bo2(16)] packed into one [128, 64] tensor
    biases = nc.dram_tensor("biases", [128, 2 * MT + WT], F32,
                            kind="ExternalInput")
    xout = nc.dram_tensor("xout", [WT, 128, ROWS], F32, kind="ExternalOutput")

    with tile.TileContext(nc) as tc:
        with (
            tc.tile_pool(name="xbuf", bufs=KT) as xpool,
            tc.tile_pool(name="bias", bufs=1) as bpool,
            tc.tile_pool(name="wfw", bufs=5) as wfpool,
            tc.tile_pool(name="wm", bufs=6) as wmpool,
            tc.tile_pool(name="wo2", bufs=WT) as wo2pool,
            tc.tile_pool(name="spec", bufs=16) as spool,
            tc.tile_pool(name="wt", bufs=16) as wtpool,
            tc.tile_pool(name="ew", bufs=8) as ewpool,
            tc.tile_pool(name="out", bufs=4) as opool,
            tc.tile_pool(name="psum", bufs=8, space="PSUM") as pspool,
        ):
            # first stage-1 weight blocks go out before x so the PE can
            # start as soon as x k-tiles stream in; x loads issue on the
            # (idle-at-start) scalar engine's DGE, weights on sync's
            # chunked so the first matmuls start after the first 512-col
            # slice lands rather than after the whole 768KB block
            wf_tiles = {}
            for j in (0, 4):                 # t=0 groups' first two blocks
                wtile = wfpool.tile([128, W], ACT, tag="wf")
                for c in range(4):
                    nc.sync.dma_start(wtile[:, bass.ts(c, W // 4)],
                                      ew1[j][:, bass.ts(c, W // 4)])
                wf_tiles[j] = wtile

            xpair = []
            for kp in range(KT // 2):
                xt = xpool.tile([128, 2 * ROWS], ACT, tag="x")
                nc.scalar.dma_start(xt[:], xin[kp])
                xpair.append(xt)

            def xslice(k):
                return xpair[k // 2][:, bass.ts(k % 2, ROWS)]

            btile = bpool.tile([128, 2 * MT + WT], F32, tag="biases")
            nc.sync.dma_start(btile[:], biases[:])

            def sb1_col(j):
                return btile[:, j:j + 1]

            def sb2_col(j):
                return btile[:, MT + j:MT + j + 1]

            def bo2_col(j):
                return btile[:, 2 * MT + j:2 * MT + j + 1]

            def elementwise(St, first):
                """complex triple product on one partition-row group."""
                qA, qB, kA, kB, vA, vB = St
                v = nc.vector
                cr = ewpool.tile([128, ROWS], F32, tag="ew")
                ci = ewpool.tile([128, ROWS], F32, tag="ew")
                t0 = ewpool.tile([128, ROWS], F32, tag="ew")
                v.tensor_mul(cr[:], kA[:], qA[:])
                v.tensor_mul(t0[:], kB[:], qB[:])
                v.tensor_add(cr[:], cr[:], t0[:])
                v.tensor_mul(ci[:], kB[:], qA[:])
                v.tensor_mul(t0[:], kA[:], qB[:])
                v.tensor_sub(ci[:], ci[:], t0[:])
                wr = wtpool.tile([128, ROWS], ACT, tag="wt")
                wi = wtpool.tile([128, ROWS], ACT, tag="wt")
                v.tensor_mul(wr[:], cr[:], vA[:])
                v.tensor_mul(t0[:], ci[:], vB[:])
                v.tensor_sub(wr[:], wr[:], t0[:])
                v.tensor_mul(wi[:], cr[:], vB[:])
                v.tensor_mul(t0[:], ci[:], vA[:])
                v.tensor_add(wi[:], wi[:], t0[:])
                if first:
                    # slot 0: A holds DC, B holds Nyquist — both real
                    v.tensor_mul(t0[0:1, :], qA[0:1, :], kA[0:1, :])
                    v.tensor_mul(wr[0:1, :], t0[0:1, :], vA[0:1, :])
                    v.tensor_mul(t0[0:1, :], qB[0:1, :], kB[0:1, :])
                    v.tensor_mul(wi[0:1, :], t0[0:1, :], vB[0:1, :])
                return wr, wi

            # ---- stage 1: S1 = x @ EW1 + b1, pipelined elementwise ----
            # The first two groups' k-loops are interleaved so every
            # arriving x pair unlocks 4 matmuls — keeps the PE fed while
            # x is still streaming in at kernel start.
            Wcat1 = [None] * ST
            for t in range(4):
                St = []
                if t == 0:
                    w0, w1 = wf_tiles.pop(0), wf_tiles.pop(4)
                    ps0 = pspool.tile([128, ROWS], F32, tag="ps")
                    ps1 = pspool.tile([128, ROWS], F32, tag="ps")
                    for k in range(KT):
                        nc.tensor.matmul(
                            ps0[:], w0[:, bass.ts(k, 128)], xslice(k),
                            start=(k == 0), stop=(k == KT - 1))
                        nc.tensor.matmul(
                            ps1[:], w1[:, bass.ts(k, 128)], xslice(k),
                            start=(k == 0), stop=(k == KT - 1))
                    for j, ps in ((0, ps0), (4, ps1)):
                        Sj = spool.tile([128, ROWS], F32, tag="spec")
                        nc.scalar.activation(Sj[:], ps[:], IDENT,
                                             bias=sb1_col(j))
                        St.append(Sj)
                bstart = 2 if t == 0 else 0
                for b in range(bstart, 6):   # qA qB kA kB vA vB row t
                    j = b * 4 + t
                    wtile = wfpool.tile([128, W], ACT, tag="wf")
                    nc.sync.dma_start(wtile[:], ew1[j])
                    ps = pspool.tile([128, ROWS], F32, tag="ps")
                    for k in range(KT):
                        nc.tensor.matmul(
                            ps[:], wtile[:, bass.ts(k, 128)], xslice(k),
                            start=(k == 0), stop=(k == KT - 1))
                    Sj = spool.tile([128, ROWS], F32, tag="spec")
                    nc.scalar.activation(Sj[:], ps[:], IDENT,
                                         bias=sb1_col(j))
                    St.append(Sj)
                wr, wi = elementwise(St, t == 0)
                Wcat1[2 * t] = wr
                Wcat1[2 * t + 1] = wi

            # ---- stage 2: S2 = Wt1 @ M12 + b2, pipelined elementwise ----
            # wop2 is preloaded during stage 2 so stage 3 never starves
            wo2_tiles = []
            Wcat2 = [None] * ST
            for t in range(4):
                St = []
                for b in range(6):
                    j = b * 4 + t
                    if len(wo2_tiles) < WT:
                        j2 = len(wo2_tiles)
                        w2 = wo2pool.tile([128, P], ACT, tag="wo2")
                        nc.scalar.dma_start(w2[:], wop2[j2])
                        wo2_tiles.append(w2)
                    wtile = wmpool.tile([128, P], ACT, tag="wm")
                    nc.sync.dma_start(wtile[:], m12[j])
                    ps = pspool.tile([128, ROWS], F32, tag="ps")
                    for s in range(ST):
                        nc.tensor.matmul(
                            ps[:], wtile[:, bass.ts(s, 128)], Wcat1[s][:],
                            start=(s == 0), stop=(s == ST - 1))
                    Sj = spool.tile([128, ROWS], F32, tag="spec")
                    nc.scalar.activation(Sj[:], ps[:], IDENT,
                                         bias=sb2_col(j))
                    St.append(Sj)
                wr, wi = elementwise(St, t == 0)
                Wcat2[2 * t] = wr
                Wcat2[2 * t + 1] = wi

            # ---- stage 3: out = Wt2 @ WoP2 + bo2 ----
            for j in range(WT):
                wtile = wo2_tiles[j]
                ps = pspool.tile([128, ROWS], F32, tag="ps")
                for s in range(ST):
                    nc.tensor.matmul(
                        ps[:], wtile[:, bass.ts(s, 128)], Wcat2[s][:],
                        start=(s == 0), stop=(s == ST - 1))
                ostage = opool.tile([128, ROWS], F32, tag="out")
                nc.vector.tensor_scalar_add(ostage[:], ps[:], bo2_col(j))
                (nc.scalar if j % 2 else nc.sync).dma_start(
                    xout[j], ostage[:])
    nc.compile()
    return nc


_MODULE_CACHE = {}


def _get_module():
    if "nc" not in _MODULE_CACHE:
        _MODULE_CACHE["nc"] = _build_module()
    return _MODULE_CACHE["nc"]


def _prepare_weight_maps(conv_w, conv_b, wq, bq, wk, bk, wv, bv, wo, bo):
    folds = [_fold_layer(conv_w[l], conv_b[l], wq[l], bq[l], wk[l], bk[l],
                         wv[l], bv[l], wo[l], bo[l]) for l in range(L)]
    EW1, Sb1, WoP1, _bo1 = folds[0]
    EW2, Sb2, WoP2, bo2 = folds[1]
    M12 = WoP1 @ EW2                               # [P, 3P], fp64
    Sb2e = _bo1 @ EW2 + Sb2                        # [3P]

    def pack(Wm, n_k, n_m):
        # [n_k*128, n_m*128] -> [n_m, 128, n_k*128] partition-contiguous
        return np.ascontiguousarray(
            Wm.reshape(n_k, 128, n_m, 128).transpose(2, 1, 0, 3)
            .reshape(n_m, 128, n_k * 128).astype(ACT_NP))

    biases = np.concatenate([
        Sb1.reshape(MT, 128).T, Sb2e.reshape(MT, 128).T,
        bo2.reshape(WT, 128).T], axis=1).astype(np.float32)
    return {
        "ew1": pack(EW1, KT, MT),
        "m12": pack(M12, ST, MT),
        "wop2": pack(WoP2, ST, WT),
        "biases": np.ascontiguousarray(biases),
    }


def _make_in_maps(inputs):
    x = np.asarray(inputs["x"], dtype=np.float32)
    wmap = _prepare_weight_maps(
        np.asarray(inputs["conv_w"]), np.asarray(inputs["conv_b"]),
        np.asarray(inputs["wq"]), np.asarray(inputs["bq"]),
        np.asarray(inputs["wk"]), np.asarray(inputs["bk"]),
        np.asarray(inputs["wv"]), np.asarray(inputs["bv"]),
        np.asarray(inputs["wo"]), np.asarray(inputs["bo"]))
    per_core = BS // NCORES
    in_maps = []
    for c in range(NCORES):
        xc = x[c * per_core:(c + 1) * per_core].reshape(ROWS, W)
        xin = np.ascontiguousarray(
            xc.reshape(ROWS, KT // 2, 2, 128).transpose(1, 3, 2, 0)
            .reshape(KT // 2, 128, 2 * ROWS).astype(ACT_NP))
        in_maps.append({"xin": xin, **wmap})
    return in_maps


def kernel(x, conv_w, conv_b, wq, bq, wk, bk, wv, bv, wo, bo):
    in_maps = _make_in_maps(dict(
        x=x, conv_w=conv_w, conv_b=conv_b, wq=wq, bq=bq, wk=wk, bk=bk,
        wv=wv, bv=bv, wo=wo, bo=bo))
    nc = _get_module()
    res = run_bass_kernel_spmd(nc, in_maps, list(range(NCORES)))

    per_core = BS // NCORES
    outs = []
    for c in range(NCORES):
        xo = res.results[c]["xout"]                    # [WT, 128, ROWS]
        outs.append(xo.transpose(2, 0, 1).reshape(per_core, CNT, W))
    return np.concatenate(outs, axis=0).astype(np.float32)
